# revision 36
# baseline (speedup 1.0000x reference)
"""Trainium2 Bass kernel for nn_EngramModule: single-query top-k memory attention
with gated residual + LayerNorm, data-parallel across 8 NeuronCores.

Contract: kernel(**inputs) takes the FULL unsharded inputs and returns the FULL
(8192, 1024) float32 output.

Per-core pipeline (1024 batch rows, 8 row-tiles of 128):
  A:  Q = h @ Wq on TensorE (bf16), staged to SBUF as bf16.
  Fused main loop over row-tiles: attention block for tile t runs back-to-back
  with the epilogue block for tile t-1, so TensorE never drains between the
  attention matmuls and the epilogue matmuls/transposes.

  Attention block (per tile, per k-slot): K/V projections as fp8e4 DoubleRow
  matmuls (2 contraction chunks per instruction, 2x bf16 MAC throughput) into
  single-bank PSUM halves for fast turnaround; DVE forms score products
  straight from PSUM; per-pair score reduce + exp (unnormalized, online);
  ScalarE expands e per slot and stages V to bf16; DVE multiplies packed bf16
  products; GpSimd tree-sums them; softmax normalization is applied once at
  the end via the reciprocal denominator.

  Epilogue block: ao transpose (bf16), mo = aoT @ Wo (bf16), mo transpose,
  gate = [h|mo] @ Wg (+bg), sigmoid via 0.5*tanh(x/2)+0.5 on ScalarE,
  aug = h + g*mo with LayerNorm stats accumulated on the fly, VectorE-only
  Newton rsqrt, scale/shift, DMA out. mo/gate PSUM chains share one 2-bank
  allocation sequentially; GpSimd takes the wide adds so DVE keeps up.

Bulk weight streams ride the ScalarE HWDGE queue; latency-critical per-tile
loads ride the SyncE queue. Activations are pre-laid-out on host (pure
transpose/reshape + dtype cast) so the contraction dim sits on SBUF
partitions.
"""

import os
import sys

import numpy as np

for _p in ("/opt/trn_rl_repo", "/root/.axon_site/_ro/trn_rl_repo"):
    if os.path.isdir(_p) and _p not in sys.path:
        sys.path.insert(0, _p)

from contextlib import ExitStack

import concourse.bacc as bacc
import concourse.mybir as mybir
import concourse.tile as tile
from concourse.bass_utils import run_bass_kernel_spmd

F32 = mybir.dt.float32
F32R = mybir.dt.float32r
BF16 = mybir.dt.bfloat16
F8 = mybir.dt.float8e4
I32 = mybir.dt.int32
AX = mybir.AxisListType
OP = mybir.AluOpType
AF = mybir.ActivationFunctionType
DR = mybir.MatmulPerfMode.DoubleRow

N_CORES = 8
B = 8192
HID = 1024
NH = 16
DH = 64
TOPK = 8
LN_EPS = 1e-5

BC = B // N_CORES          # rows per core = 1024
NT = BC // 128             # row-tiles per core = 8
NIC = HID // 128           # 128-row contraction chunks = 8
NJH = HID // 512           # 512-wide output halves = 2
SCALE = DH ** -0.5
RSQRT_MAGIC = 0x5F3759DF

# Set by test.py to collect a profile; grading path leaves this off.
TRACE = False

_CACHE = {}


def _build(nt=NT):
    nc = bacc.Bacc("TRN2", target_bir_lowering=False, debug=False,
                   num_devices=N_CORES)

    # ---- DRAM parameters (per-core shard, host-prepped layouts) ----
    h_d = nc.declare_dram_parameter("h", [nt, 128, HID], F32, isOutput=False)
    mkT_d = nc.declare_dram_parameter("mkT", [nt, TOPK, 128, NIC, 128], F8, isOutput=False)
    mvT_d = nc.declare_dram_parameter("mvT", [nt, TOPK, 128, NIC, 128], F8, isOutput=False)
    hTb_d = nc.declare_dram_parameter("hTb", [nt, 128, NIC, 128], BF16, isOutput=False)
    wq_d = nc.declare_dram_parameter("Wq", [128, NIC, HID], BF16, isOutput=False)
    wk_d = nc.declare_dram_parameter("Wk", [128, NIC, HID], F8, isOutput=False)
    wv_d = nc.declare_dram_parameter("Wv", [128, NIC, HID], F8, isOutput=False)
    wo_d = nc.declare_dram_parameter("Wo", [128, NIC, HID], BF16, isOutput=False)
    wg_d = nc.declare_dram_parameter("Wg", [128, 2 * NIC, HID], BF16, isOutput=False)
    bgb_d = nc.declare_dram_parameter("bgB", [128, HID], F32, isOutput=False)
    eye_d = nc.declare_dram_parameter("eye", [128, 128], BF16, isOutput=False)
    lng_d = nc.declare_dram_parameter("lngB", [128, HID], BF16, isOutput=False)
    lnb_d = nc.declare_dram_parameter("lnbB", [128, HID], BF16, isOutput=False)
    out_d = nc.declare_dram_parameter("out", [nt, 128, HID], F32, isOutput=True)

    def load_w(tile_sb, dram, nchunk):
        for ic in range(nchunk):
            nc.scalar.dma_start(tile_sb[:, ic, :], dram.ap()[:, ic, :])

    with ExitStack() as octx:
        tc = octx.enter_context(tile.TileContext(nc))

        pers = octx.enter_context(tc.tile_pool(name="pers", bufs=1))
        sum_all = pers.tile([128, nt], F32, tag="sum_all")
        ss_all = pers.tile([128, nt], F32, tag="ss_all")
        hT_all = pers.tile([128, nt, NIC, 128], BF16, tag="hT")
        eye_sb = pers.tile([128, 128], BF16, tag="eye")
        bgb_sb = pers.tile([128, HID], F32, tag="bgb")
        lng_sb = pers.tile([128, HID], BF16, tag="lng")
        lnb_sb = pers.tile([128, HID], BF16, tag="lnb")

        wp = octx.enter_context(tc.tile_pool(name="weights", bufs=1))
        wo_sb = wp.tile([128, NIC, HID], BF16, tag="wo")
        wg_sb = wp.tile([128, 2 * NIC, HID], BF16, tag="wg")
        wk_sb = wp.tile([128, NIC, HID], F8, tag="wk")
        wv_sb = wp.tile([128, NIC, HID], F8, tag="wv")

        qp = octx.enter_context(tc.tile_pool(name="q_all", bufs=1))
        q_all = qp.tile([128, nt, HID], BF16, tag="q_all")

        aop = octx.enter_context(tc.tile_pool(name="ao", bufs=2))

        # ================= phase A: Q projection =================
        with ExitStack() as actx:
            wqp = actx.enter_context(tc.tile_pool(name="wq", bufs=1))
            qps = actx.enter_context(tc.tile_pool(name="q_ps", bufs=2, space="PSUM"))
            wq_sb = wqp.tile([128, NIC, HID], BF16, tag="wq")
            load_w(wq_sb, wq_d, NIC)
            load_w(wk_sb, wk_d, NIC)
            load_w(wv_sb, wv_d, NIC)
            for t in range(nt):
                nc.sync.dma_start(hT_all[:, t], hTb_d.ap()[t])
            nc.sync.dma_start(eye_sb[:], eye_d.ap())
            nc.sync.dma_start(bgb_sb[:], bgb_d.ap())
            nc.sync.dma_start(lng_sb[:], lng_d.ap())
            nc.sync.dma_start(lnb_sb[:], lnb_d.ap())
            for t in range(nt):
                q_ps = qps.tile([128, HID], F32, tag="qps")
                for ic in range(NIC):
                    for jh in range(NJH):
                        nc.tensor.matmul(
                            q_ps[:, jh * 512:(jh + 1) * 512],
                            hT_all[:, t, ic, :],
                            wq_sb[:, ic, jh * 512:(jh + 1) * 512],
                            start=(ic == 0), stop=(ic == NIC - 1),
                        )
                nc.scalar.copy(q_all[:, t, :], q_ps[:])

        # ============ fused main loop: B(t) then C(t-1) ============
        load_w(wo_sb, wo_d, NIC)
        load_w(wg_sb, wg_d, 2 * NIC)

        mp = octx.enter_context(tc.tile_pool(name="mkv", bufs=3))
        kvps = octx.enter_context(tc.tile_pool(name="kv_ps", bufs=1, space="PSUM"))
        papl = octx.enter_context(tc.tile_pool(name="p_all", bufs=1))
        vpsb = octx.enter_context(tc.tile_pool(name="vp_sb", bufs=1))
        sc = octx.enter_context(tc.tile_pool(name="scr", bufs=2))
        ex = octx.enter_context(tc.tile_pool(name="eexp", bufs=2))
        pr = octx.enter_context(tc.tile_pool(name="prod", bufs=1))
        cstr = octx.enter_context(tc.tile_pool(name="c_str", bufs=2))
        csb = octx.enter_context(tc.tile_pool(name="c_sb", bufs=2))
        stp = octx.enter_context(tc.tile_pool(name="stats", bufs=2))
        tps = octx.enter_context(tc.tile_pool(name="tp_ps", bufs=1, space="PSUM"))
        cps = octx.enter_context(tc.tile_pool(name="c_ps", bufs=1, space="PSUM"))

        preload = {}
        for (pt, pk) in ((0, 0),):
            a = mp.tile([128, NIC, 128], F8, tag="mkT")
            nc.sync.dma_start(a[:], mkT_d.ap()[pt, pk])
            b_ = mp.tile([128, NIC, 128], F8, tag="mvT")
            nc.sync.dma_start(b_[:], mvT_d.ap()[pt, pk])
            preload[(pt, pk)] = (a, b_)

        ao_tiles = {}

        def b_block(t, cgen=None):
            p_all = papl.tile([128, TOPK, HID], BF16, tag="p_all")
            vp_sb = vpsb.tile([128, TOPK, HID], BF16, tag="vp_sb")
            s_all = sc.tile([128, TOPK, NH], F32, tag="s_all")
            e_all = sc.tile([128, TOPK, NH], F32, tag="e_all")
            prod = pr.tile([128, TOPK, HID], BF16, tag="prod")
            for k in range(TOPK):
                if cgen is not None and k >= 1:
                    next(cgen, None)
                if (t, k) in preload:
                    mkT, mvT = preload[(t, k)]
                else:
                    mkT = mp.tile([128, NIC, 128], F8, tag="mkT")
                    nc.sync.dma_start(mkT[:], mkT_d.ap()[t, k])
                    mvT = mp.tile([128, NIC, 128], F8, tag="mvT")
                    nc.sync.dma_start(mvT[:], mvT_d.ap()[t, k])

                # fp8 DoubleRow chains into single-bank PSUM halves
                for jh in range(NJH):
                    kp_h = kvps.tile([128, 512], F32, tag=f"kp{jh}")
                    for c2 in range(NIC // 2):
                        nc.tensor.matmul(
                            kp_h[:],
                            mkT[:, 2 * c2:2 * c2 + 2, :],
                            wk_sb[:, 2 * c2:2 * c2 + 2, jh * 512:(jh + 1) * 512],
                            start=(c2 == 0), stop=(c2 == NIC // 2 - 1),
                            perf_mode=DR,
                        )
                    nc.vector.tensor_mul(
                        p_all[:, k, jh * 512:(jh + 1) * 512], kp_h[:],
                        q_all[:, t, jh * 512:(jh + 1) * 512])
                for jh in range(NJH):
                    vp_h = kvps.tile([128, 512], F32, tag=f"vp{jh}")
                    for c2 in range(NIC // 2):
                        nc.tensor.matmul(
                            vp_h[:],
                            mvT[:, 2 * c2:2 * c2 + 2, :],
                            wv_sb[:, 2 * c2:2 * c2 + 2, jh * 512:(jh + 1) * 512],
                            start=(c2 == 0), stop=(c2 == NIC // 2 - 1),
                            perf_mode=DR,
                        )
                    nc.scalar.copy(vp_sb[:, k, jh * 512:(jh + 1) * 512], vp_h[:])

                if k % 2 == 1:
                    # scores for this slot pair; online (unnormalized) e
                    nc.vector.reduce_sum(
                        s_all[:, k - 1:k + 1, :],
                        p_all[:, k - 1:k + 1, :].rearrange(
                            "p k (h d) -> p k h d", h=NH),
                        axis=AX.X)
                    nc.scalar.activation(
                        e_all[:, k - 1:k + 1, :], s_all[:, k - 1:k + 1, :],
                        AF.Exp, scale=SCALE)
                    for kk in (k - 1, k):
                        eexp = ex.tile([128, HID], BF16, tag="eexp")
                        nc.scalar.copy(
                            eexp[:].rearrange("p (h d) -> p h d", h=NH),
                            e_all[:, kk, :].unsqueeze(2).broadcast_to(
                                [128, NH, DH]))
                        nc.vector.tensor_mul(
                            prod[:, kk, :], vp_sb[:, kk, :], eexp[:])
                    nc.gpsimd.tensor_add(
                        prod[:, k - 1, :], prod[:, k - 1, :], prod[:, k, :])

            den = sc.tile([128, NH], F32, tag="den")
            nc.vector.reduce_sum(
                den[:], e_all[:].rearrange("p k h -> p h k"), axis=AX.X)
            rden = sc.tile([128, NH], F32, tag="rden")
            nc.vector.reciprocal(rden[:], den[:])
            nc.gpsimd.tensor_add(prod[:, 0, :], prod[:, 0, :], prod[:, 2, :])
            nc.gpsimd.tensor_add(prod[:, 4, :], prod[:, 4, :], prod[:, 6, :])
            nc.gpsimd.tensor_add(prod[:, 0, :], prod[:, 0, :], prod[:, 4, :])
            ao = aop.tile([128, HID], BF16, tag="ao")
            ao_tiles[t] = ao
            rden_bc = rden[:].unsqueeze(2).broadcast_to([128, NH, DH])
            nc.vector.tensor_tensor(
                ao[:].rearrange("p (h d) -> p h d", h=NH),
                prod[:, 0, :].rearrange("p (h d) -> p h d", h=NH),
                rden_bc, op=OP.mult)

        def c_block(t):
            """Epilogue for tile t, yielded in 8 pieces so the caller can
            interleave them between the next tile's attention k-steps."""
            ao = ao_tiles.pop(t)
            h_sb = cstr.tile([128, HID], F32, tag="h_c")
            nc.sync.dma_start(h_sb[:], h_d.ap()[t])

            at_ps = tps.tile([128, NIC, 128], BF16, tag="tp_ps")
            for ic in range(NIC):
                nc.tensor.transpose(
                    at_ps[:, ic, :], ao[:, ic * 128:(ic + 1) * 128], eye_sb[:])
            atT_sb = csb.tile([128, NIC, 128], BF16, tag="tT")
            nc.scalar.copy(atT_sb[:], at_ps[:])
            yield

            mo_ps = cps.tile([128, HID], F32, tag="c_ps")
            for ic in range(NIC):
                for jh in range(NJH):
                    nc.tensor.matmul(
                        mo_ps[:, jh * 512:(jh + 1) * 512],
                        atT_sb[:, ic, :],
                        wo_sb[:, ic, jh * 512:(jh + 1) * 512],
                        start=(ic == 0), stop=(ic == NIC - 1),
                    )
            yield

            mo_sb = csb.tile([128, HID], BF16, tag="mo")
            nc.scalar.copy(mo_sb[:], mo_ps[:])
            moT_ps = tps.tile([128, NIC, 128], BF16, tag="tp_ps")
            for ic in range(NIC):
                nc.tensor.transpose(
                    moT_ps[:, ic, :], mo_sb[:, ic * 128:(ic + 1) * 128],
                    eye_sb[:])
            moT_sb = csb.tile([128, NIC, 128], BF16, tag="tT")
            nc.scalar.copy(moT_sb[:], moT_ps[:])
            yield

            # gate: both halves as one PSUM accumulation run (shared banks)
            g_ps = cps.tile([128, HID], F32, tag="c_ps")
            for ic in range(NIC):
                for jh in range(NJH):
                    sl = slice(jh * 512, (jh + 1) * 512)
                    nc.tensor.matmul(
                        g_ps[:, sl], hT_all[:, t, ic, :], wg_sb[:, ic, sl],
                        start=(ic == 0), stop=False)
            yield

            for ic in range(NIC):
                for jh in range(NJH):
                    sl = slice(jh * 512, (jh + 1) * 512)
                    nc.tensor.matmul(
                        g_ps[:, sl], moT_sb[:, ic, :], wg_sb[:, NIC + ic, sl],
                        start=False, stop=(ic == NIC - 1))
            yield

            gb_sb = csb.tile([128, HID], F32, tag="gb")
            nc.vector.tensor_add(gb_sb[:], g_ps[:], bgb_sb[:])
            # sigmoid(x) = 0.5*tanh(x/2) + 0.5 (tanh shares ACT set w/ exp)
            nc.scalar.activation(gb_sb[:], gb_sb[:], AF.Tanh, scale=0.5)

            # host pre-halved Wo, so mo_sb = 0.5*mo:
            # aug = h + g*mo = (h + mo_sb) + mo_sb*tanh  (plain adds/mults
            # keep GpSimd eligible -- Pool has no tensor_scalar_ptr ops)
            v_sb = csb.tile([128, HID], F32, tag="v")
            nc.gpsimd.tensor_add(h_sb[:], h_sb[:], mo_sb[:])
            nc.gpsimd.tensor_mul(v_sb[:], mo_sb[:], gb_sb[:])
            yield

            nc.vector.scalar_tensor_tensor(
                h_sb[:], h_sb[:], 0.0, v_sb[:], op0=OP.add, op1=OP.add,
                accum_out=sum_all[:, t:t + 1])
            # square's tensor output is scrap; we only keep the accumulator
            nc.scalar.activation(
                v_sb[:], h_sb[:], AF.Square, accum_out=ss_all[:, t:t + 1])
            yield

            # ---- LayerNorm finalize, per tile, VectorE only ----
            mean = stp.tile([128, 1], F32, tag="mean")
            nc.vector.tensor_scalar_mul(mean[:], sum_all[:, t:t + 1], 1.0 / HID)
            m2 = stp.tile([128, 1], F32, tag="m2")
            nc.vector.tensor_mul(m2[:], mean[:], mean[:])
            nc.vector.tensor_scalar_add(m2[:], m2[:], -LN_EPS)
            vpe = stp.tile([128, 1], F32, tag="vpe")
            nc.vector.scalar_tensor_tensor(
                vpe[:], ss_all[:, t:t + 1], 1.0 / HID, m2[:],
                op0=OP.mult, op1=OP.subtract)
            # rstd = 1/sqrt(vpe): quake init + 3 Newton iterations
            y = stp.tile([128, 1], F32, tag="y")
            yi = y[:].bitcast(I32)
            nc.vector.tensor_scalar(
                yi, vpe[:].bitcast(I32), 1, None,
                op0=OP.logical_shift_right)
            nc.vector.tensor_scalar(
                yi, yi, -RSQRT_MAGIC, -1,
                op0=OP.add, op1=OP.mult)
            yy = stp.tile([128, 1], F32, tag="yy")
            hw = stp.tile([128, 1], F32, tag="hw")
            for _ in range(3):
                nc.vector.tensor_mul(yy[:], y[:], y[:])
                nc.vector.tensor_mul(yy[:], yy[:], vpe[:])
                nc.vector.tensor_scalar(
                    hw[:], yy[:], -0.5, 1.5, op0=OP.mult, op1=OP.add)
                nc.vector.tensor_mul(y[:], y[:], hw[:])

            # yout = (aug - mean)*rstd*lng + lnb
            nc.vector.scalar_tensor_tensor(
                h_sb[:], h_sb[:], mean[:], lng_sb[:],
                op0=OP.subtract, op1=OP.mult)
            yo_sb = cstr.tile([128, HID], F32, tag="h_c")
            nc.vector.scalar_tensor_tensor(
                yo_sb[:], h_sb[:], y[:], lnb_sb[:],
                op0=OP.mult, op1=OP.add)
            nc.sync.dma_start(out_d.ap()[t], yo_sb[:])

        cgen = None
        for t in range(nt):
            cgen = c_block(t - 1) if t >= 1 else None
            b_block(t, cgen)
            if cgen is not None:
                for _ in cgen:
                    pass
        for _ in c_block(nt - 1):
            pass

    nc.compile()
    return nc


def _prep_core(hs, mk, mv, nt):
    """Host-side lossless layout prep for one core's shard."""
    hT = np.ascontiguousarray(
        hs.reshape(nt, 128, NIC, 128).transpose(0, 3, 2, 1))      # [t,p,ic,b]
    h = np.ascontiguousarray(hs.reshape(nt, 128, HID))
    mkT = np.ascontiguousarray(
        mk.reshape(nt, 128, TOPK, NIC, 128).transpose(0, 2, 4, 3, 1))
    mvT = np.ascontiguousarray(
        mv.reshape(nt, 128, TOPK, NIC, 128).transpose(0, 2, 4, 3, 1))
    return hT, h, mkT, mvT


def kernel(**inputs):
    hs = np.asarray(inputs["hidden_state"], dtype=np.float32)
    mk = np.asarray(inputs["memory_keys"], dtype=np.float32)
    mv = np.asarray(inputs["memory_values"], dtype=np.float32)

    import ml_dtypes
    bf = ml_dtypes.bfloat16
    f8 = ml_dtypes.float8_e4m3
    wq = np.ascontiguousarray(
        np.asarray(inputs["Wq"], np.float32).reshape(NIC, 128, HID).transpose(1, 0, 2)).astype(bf)
    wk = np.ascontiguousarray(
        np.asarray(inputs["Wk"], np.float32).reshape(NIC, 128, HID).transpose(1, 0, 2)).astype(f8)
    wv = np.ascontiguousarray(
        np.asarray(inputs["Wv"], np.float32).reshape(NIC, 128, HID).transpose(1, 0, 2)).astype(f8)
    # Wo pre-halved so mo_sb = 0.5*mo on chip; Wg's mo-half doubled to
    # compensate (both exact power-of-2 scalings in bf16)
    wo = np.ascontiguousarray(
        (np.asarray(inputs["Wo"], np.float32) * 0.5).reshape(NIC, 128, HID).transpose(1, 0, 2)).astype(bf)
    wg_f = np.asarray(inputs["Wg"], np.float32).copy()
    wg_f[HID:] *= 2.0
    wg = np.ascontiguousarray(
        wg_f.reshape(2 * NIC, 128, HID).transpose(1, 0, 2)).astype(bf)
    bgb = np.ascontiguousarray(
        np.broadcast_to(np.asarray(inputs["bg"], np.float32), (128, HID)))
    lng = np.ascontiguousarray(
        np.broadcast_to(np.asarray(inputs["ln_g"], np.float32).astype(bf), (128, HID)))
    lnb = np.ascontiguousarray(
        np.broadcast_to(np.asarray(inputs["ln_b"], np.float32).astype(bf), (128, HID)))
    eye = np.eye(128, dtype=np.float32).astype(bf)

    if "nc" not in _CACHE:
        _CACHE["nc"] = _build(NT)
    nc = _CACHE["nc"]

    in_maps = []
    for c in range(N_CORES):
        sl = slice(c * BC, (c + 1) * BC)
        hT, h, mkT, mvT = _prep_core(hs[sl], mk[sl], mv[sl], NT)
        in_maps.append({
            "hTb": hT.astype(bf), "h": h,
            "mkT": mkT.astype(f8), "mvT": mvT.astype(f8),
            "Wq": wq, "Wk": wk, "Wv": wv, "Wo": wo, "Wg": wg,
            "bgB": bgb, "eye": eye, "lngB": lng, "lnbB": lnb,
        })

    res = run_bass_kernel_spmd(nc, in_maps, core_ids=list(range(N_CORES)),
                               trace=TRACE)
    kernel.last_result = res
    out = np.concatenate(
        [r["out"].reshape(BC, HID) for r in res.results], axis=0)
    return out


kernel.last_result = None


# revision 39
# speedup vs baseline: 1.1759x; 1.1759x over previous
"""Trainium2 Bass kernel for nn_EngramModule: single-query top-k memory attention
with gated residual + LayerNorm, data-parallel across 8 NeuronCores.

Contract: kernel(**inputs) takes the FULL unsharded inputs and returns the FULL
(8192, 1024) float32 output.

Per-core pipeline (1024 batch rows, 8 row-tiles of 128):
  A:  Q = h @ Wq on TensorE (bf16), staged to SBUF as bf16.
  Fused main loop over row-tiles: attention block for tile t runs back-to-back
  with the epilogue block for tile t-1, so TensorE never drains between the
  attention matmuls and the epilogue matmuls/transposes.

  Attention block (per tile, per k-slot): K/V projections as fp8e4 DoubleRow
  matmuls (2 contraction chunks per instruction, 2x bf16 MAC throughput) into
  single-bank PSUM halves for fast turnaround; DVE forms score products
  straight from PSUM; per-pair score reduce + exp (unnormalized, online);
  ScalarE expands e per slot and stages V to bf16; DVE multiplies packed bf16
  products; GpSimd tree-sums them; softmax normalization is applied once at
  the end via the reciprocal denominator.

  Epilogue block: ao transpose (bf16), mo = aoT @ Wo (bf16), mo transpose,
  gate = [h|mo] @ Wg (+bg), sigmoid via 0.5*tanh(x/2)+0.5 on ScalarE,
  aug = h + g*mo with LayerNorm stats accumulated on the fly, VectorE-only
  Newton rsqrt, scale/shift, DMA out. mo/gate PSUM chains share one 2-bank
  allocation sequentially; GpSimd takes the wide adds so DVE keeps up.

Bulk weight streams ride the ScalarE HWDGE queue; latency-critical per-tile
loads ride the SyncE queue. Activations are pre-laid-out on host (pure
transpose/reshape + dtype cast) so the contraction dim sits on SBUF
partitions.
"""

import os
import sys

import numpy as np

for _p in ("/opt/trn_rl_repo", "/root/.axon_site/_ro/trn_rl_repo"):
    if os.path.isdir(_p) and _p not in sys.path:
        sys.path.insert(0, _p)

from contextlib import ExitStack

import concourse.bacc as bacc
import concourse.mybir as mybir
import concourse.tile as tile
from concourse.bass_utils import run_bass_kernel_spmd

F32 = mybir.dt.float32
F32R = mybir.dt.float32r
BF16 = mybir.dt.bfloat16
F8 = mybir.dt.float8e4
I32 = mybir.dt.int32
AX = mybir.AxisListType
OP = mybir.AluOpType
AF = mybir.ActivationFunctionType
DR = mybir.MatmulPerfMode.DoubleRow

N_CORES = 8
B = 8192
HID = 1024
NH = 16
DH = 64
TOPK = 8
LN_EPS = 1e-5

BC = B // N_CORES          # rows per core = 1024
NT = BC // 128             # row-tiles per core = 8
NIC = HID // 128           # 128-row contraction chunks = 8
NJH = HID // 512           # 512-wide output halves = 2
SCALE = DH ** -0.5
RSQRT_MAGIC = 0x5F3759DF

# Set by test.py to collect a profile; grading path leaves this off.
TRACE = False

_CACHE = {}


def _build(nt=NT):
    nc = bacc.Bacc("TRN2", target_bir_lowering=False, debug=False,
                   num_devices=N_CORES)

    # ---- DRAM parameters (per-core shard, host-prepped layouts) ----
    h_d = nc.declare_dram_parameter("h", [nt, 128, HID], F32, isOutput=False)
    mkT_d = nc.declare_dram_parameter("mkT", [nt, TOPK, 128, NIC, 128], F8, isOutput=False)
    mvT_d = nc.declare_dram_parameter("mvT", [nt, TOPK, 128, NIC, 128], F8, isOutput=False)
    hTb_d = nc.declare_dram_parameter("hTb", [nt, 128, NIC, 128], BF16, isOutput=False)
    wq_d = nc.declare_dram_parameter("Wq", [128, NIC, HID], BF16, isOutput=False)
    wk_d = nc.declare_dram_parameter("Wk", [128, NIC, HID], F8, isOutput=False)
    wv_d = nc.declare_dram_parameter("Wv", [128, NIC, HID], F8, isOutput=False)
    wo_d = nc.declare_dram_parameter("Wo", [128, NIC, HID], BF16, isOutput=False)
    wg_d = nc.declare_dram_parameter("Wg", [128, 2 * NIC, HID], BF16, isOutput=False)
    bgb_d = nc.declare_dram_parameter("bgB", [128, HID], F32, isOutput=False)
    eye_d = nc.declare_dram_parameter("eye", [128, 128], BF16, isOutput=False)
    lng_d = nc.declare_dram_parameter("lngB", [128, HID], BF16, isOutput=False)
    lnb_d = nc.declare_dram_parameter("lnbB", [128, HID], BF16, isOutput=False)
    out_d = nc.declare_dram_parameter("out", [nt, 128, HID], F32, isOutput=True)

    def load_w(tile_sb, dram, nchunk):
        for ic in range(nchunk):
            nc.scalar.dma_start(tile_sb[:, ic, :], dram.ap()[:, ic, :])

    with ExitStack() as octx:
        tc = octx.enter_context(tile.TileContext(nc))

        pers = octx.enter_context(tc.tile_pool(name="pers", bufs=1))
        sum_all = pers.tile([128, nt], F32, tag="sum_all")
        ss_all = pers.tile([128, nt], F32, tag="ss_all")
        hT_all = pers.tile([128, nt, NIC, 128], BF16, tag="hT")
        eye_sb = pers.tile([128, 128], BF16, tag="eye")
        bgb_sb = pers.tile([128, HID], F32, tag="bgb")
        lng_sb = pers.tile([128, HID], BF16, tag="lng")
        lnb_sb = pers.tile([128, HID], BF16, tag="lnb")

        wp = octx.enter_context(tc.tile_pool(name="weights", bufs=1))
        wo_sb = wp.tile([128, NIC, HID], BF16, tag="wo")
        wg_sb = wp.tile([128, 2 * NIC, HID], BF16, tag="wg")
        wk_sb = wp.tile([128, NIC, HID], F8, tag="wk")
        wv_sb = wp.tile([128, NIC, HID], F8, tag="wv")

        qp = octx.enter_context(tc.tile_pool(name="q_all", bufs=1))
        q_all = qp.tile([128, nt, HID], BF16, tag="q_all")

        aop = octx.enter_context(tc.tile_pool(name="ao", bufs=2))

        # ================= phase A: Q projection =================
        with ExitStack() as actx:
            wqp = actx.enter_context(tc.tile_pool(name="wq", bufs=1))
            qps = actx.enter_context(tc.tile_pool(name="q_ps", bufs=2, space="PSUM"))
            wq_sb = wqp.tile([128, NIC, HID], BF16, tag="wq")
            load_w(wq_sb, wq_d, NIC)
            load_w(wk_sb, wk_d, NIC)
            load_w(wv_sb, wv_d, NIC)
            for t in range(nt):
                nc.sync.dma_start(hT_all[:, t], hTb_d.ap()[t])
            nc.sync.dma_start(eye_sb[:], eye_d.ap())
            nc.sync.dma_start(bgb_sb[:], bgb_d.ap())
            nc.sync.dma_start(lng_sb[:], lng_d.ap())
            nc.sync.dma_start(lnb_sb[:], lnb_d.ap())
            for t in range(nt):
                q_ps = qps.tile([128, HID], F32, tag="qps")
                for ic in range(NIC):
                    for jh in range(NJH):
                        nc.tensor.matmul(
                            q_ps[:, jh * 512:(jh + 1) * 512],
                            hT_all[:, t, ic, :],
                            wq_sb[:, ic, jh * 512:(jh + 1) * 512],
                            start=(ic == 0), stop=(ic == NIC - 1),
                        )
                nc.scalar.copy(q_all[:, t, :], q_ps[:])

        # ============ fused main loop: B(t) then C(t-1) ============
        load_w(wo_sb, wo_d, NIC)
        load_w(wg_sb, wg_d, 2 * NIC)

        mp = octx.enter_context(tc.tile_pool(name="mkv", bufs=3))
        kvps = octx.enter_context(tc.tile_pool(name="kv_ps", bufs=1, space="PSUM"))
        papl = octx.enter_context(tc.tile_pool(name="p_all", bufs=1))
        vpsb = octx.enter_context(tc.tile_pool(name="vp_sb", bufs=1))
        sc = octx.enter_context(tc.tile_pool(name="scr", bufs=2))
        ex = octx.enter_context(tc.tile_pool(name="eexp", bufs=2))
        pr = octx.enter_context(tc.tile_pool(name="prod", bufs=1))
        cstr = octx.enter_context(tc.tile_pool(name="c_str", bufs=2))
        csb = octx.enter_context(tc.tile_pool(name="c_sb", bufs=2))
        stp = octx.enter_context(tc.tile_pool(name="stats", bufs=2))
        tps = octx.enter_context(tc.tile_pool(name="tp_ps", bufs=1, space="PSUM"))
        cps = octx.enter_context(tc.tile_pool(name="c_ps", bufs=1, space="PSUM"))

        preload = {}
        for (pt, pk) in ((0, 0),):
            a = mp.tile([128, NIC, 128], F8, tag="mkT")
            nc.sync.dma_start(a[:], mkT_d.ap()[pt, pk])
            b_ = mp.tile([128, NIC, 128], F8, tag="mvT")
            nc.sync.dma_start(b_[:], mvT_d.ap()[pt, pk])
            preload[(pt, pk)] = (a, b_)

        ao_tiles = {}

        def b_block(t, cgen=None):
            p_all = papl.tile([128, TOPK, HID], BF16, tag="p_all")
            vp_sb = vpsb.tile([128, TOPK, HID], BF16, tag="vp_sb")
            s_all = sc.tile([128, TOPK, NH], F32, tag="s_all")
            e_all = sc.tile([128, TOPK, NH], F32, tag="e_all")
            prod = pr.tile([128, TOPK, HID], BF16, tag="prod")
            for k in range(TOPK):
                if cgen is not None and 1 <= k <= 5:
                    next(cgen, None)
                if (t, k) in preload:
                    mkT, mvT = preload[(t, k)]
                else:
                    mkT = mp.tile([128, NIC, 128], F8, tag="mkT")
                    nc.sync.dma_start(mkT[:], mkT_d.ap()[t, k])
                    mvT = mp.tile([128, NIC, 128], F8, tag="mvT")
                    nc.sync.dma_start(mvT[:], mvT_d.ap()[t, k])

                # fp8 DoubleRow chains into single-bank PSUM halves
                for jh in range(NJH):
                    kp_h = kvps.tile([128, 512], F32, tag=f"kp{jh}")
                    for c2 in range(NIC // 2):
                        nc.tensor.matmul(
                            kp_h[:],
                            mkT[:, 2 * c2:2 * c2 + 2, :],
                            wk_sb[:, 2 * c2:2 * c2 + 2, jh * 512:(jh + 1) * 512],
                            start=(c2 == 0), stop=(c2 == NIC // 2 - 1),
                            perf_mode=DR,
                        )
                    nc.vector.tensor_mul(
                        p_all[:, k, jh * 512:(jh + 1) * 512], kp_h[:],
                        q_all[:, t, jh * 512:(jh + 1) * 512])
                for jh in range(NJH):
                    vp_h = kvps.tile([128, 512], F32, tag=f"vp{jh}")
                    for c2 in range(NIC // 2):
                        nc.tensor.matmul(
                            vp_h[:],
                            mvT[:, 2 * c2:2 * c2 + 2, :],
                            wv_sb[:, 2 * c2:2 * c2 + 2, jh * 512:(jh + 1) * 512],
                            start=(c2 == 0), stop=(c2 == NIC // 2 - 1),
                            perf_mode=DR,
                        )
                    nc.scalar.copy(vp_sb[:, k, jh * 512:(jh + 1) * 512], vp_h[:])

                if k % 2 == 1:
                    # scores for this slot pair; online (unnormalized) e
                    nc.vector.reduce_sum(
                        s_all[:, k - 1:k + 1, :],
                        p_all[:, k - 1:k + 1, :].rearrange(
                            "p k (h d) -> p k h d", h=NH),
                        axis=AX.X)
                    nc.scalar.activation(
                        e_all[:, k - 1:k + 1, :], s_all[:, k - 1:k + 1, :],
                        AF.Exp, scale=SCALE)
                    for kk in (k - 1, k):
                        eexp = ex.tile([128, HID], BF16, tag="eexp")
                        nc.scalar.copy(
                            eexp[:].rearrange("p (h d) -> p h d", h=NH),
                            e_all[:, kk, :].unsqueeze(2).broadcast_to(
                                [128, NH, DH]))
                        nc.vector.tensor_mul(
                            prod[:, kk, :], vp_sb[:, kk, :], eexp[:])
                    nc.gpsimd.tensor_add(
                        prod[:, k - 1, :], prod[:, k - 1, :], prod[:, k, :])

            den = sc.tile([128, NH], F32, tag="den")
            nc.vector.reduce_sum(
                den[:], e_all[:].rearrange("p k h -> p h k"), axis=AX.X)
            rden = sc.tile([128, NH], F32, tag="rden")
            nc.vector.reciprocal(rden[:], den[:])
            nc.gpsimd.tensor_add(prod[:, 0, :], prod[:, 0, :], prod[:, 2, :])
            nc.gpsimd.tensor_add(prod[:, 4, :], prod[:, 4, :], prod[:, 6, :])
            nc.gpsimd.tensor_add(prod[:, 0, :], prod[:, 0, :], prod[:, 4, :])
            ao = aop.tile([128, HID], BF16, tag="ao")
            ao_tiles[t] = ao
            rden_bc = rden[:].unsqueeze(2).broadcast_to([128, NH, DH])
            nc.vector.tensor_tensor(
                ao[:].rearrange("p (h d) -> p h d", h=NH),
                prod[:, 0, :].rearrange("p (h d) -> p h d", h=NH),
                rden_bc, op=OP.mult)

        def c_block(t):
            """Epilogue for tile t, yielded in 8 pieces so the caller can
            interleave them between the next tile's attention k-steps."""
            ao = ao_tiles.pop(t)
            h_sb = cstr.tile([128, HID], F32, tag="h_c")
            nc.sync.dma_start(h_sb[:], h_d.ap()[t])

            at_ps = tps.tile([128, NIC, 128], BF16, tag="tp_ps")
            for ic in range(NIC):
                nc.tensor.transpose(
                    at_ps[:, ic, :], ao[:, ic * 128:(ic + 1) * 128], eye_sb[:])
            atT_sb = csb.tile([128, NIC, 128], BF16, tag="tT")
            nc.scalar.copy(atT_sb[:], at_ps[:])
            yield

            mo_ps = cps.tile([128, HID], F32, tag="c_ps")
            for ic in range(NIC):
                for jh in range(NJH):
                    nc.tensor.matmul(
                        mo_ps[:, jh * 512:(jh + 1) * 512],
                        atT_sb[:, ic, :],
                        wo_sb[:, ic, jh * 512:(jh + 1) * 512],
                        start=(ic == 0), stop=(ic == NIC - 1),
                    )
            yield

            mo_sb = csb.tile([128, HID], BF16, tag="mo")
            nc.scalar.copy(mo_sb[:], mo_ps[:])
            moT_ps = tps.tile([128, NIC, 128], BF16, tag="tp_ps")
            for ic in range(NIC):
                nc.tensor.transpose(
                    moT_ps[:, ic, :], mo_sb[:, ic * 128:(ic + 1) * 128],
                    eye_sb[:])
            moT_sb = csb.tile([128, NIC, 128], BF16, tag="tT")
            nc.scalar.copy(moT_sb[:], moT_ps[:])
            yield

            # gate: both halves as one PSUM accumulation run (shared banks)
            g_ps = cps.tile([128, HID], F32, tag="c_ps")
            for ic in range(NIC):
                for jh in range(NJH):
                    sl = slice(jh * 512, (jh + 1) * 512)
                    nc.tensor.matmul(
                        g_ps[:, sl], hT_all[:, t, ic, :], wg_sb[:, ic, sl],
                        start=(ic == 0), stop=False)
            yield

            for ic in range(NIC):
                for jh in range(NJH):
                    sl = slice(jh * 512, (jh + 1) * 512)
                    nc.tensor.matmul(
                        g_ps[:, sl], moT_sb[:, ic, :], wg_sb[:, NIC + ic, sl],
                        start=False, stop=(ic == NIC - 1))
            yield  # --- tensor-side done; epilogue ops follow the B-tail ---

            gb_sb = csb.tile([128, HID], F32, tag="gb")
            nc.vector.tensor_add(gb_sb[:], g_ps[:], bgb_sb[:])
            # sigmoid(x) = 0.5*tanh(x/2) + 0.5 (tanh shares ACT set w/ exp)
            nc.scalar.activation(gb_sb[:], gb_sb[:], AF.Tanh, scale=0.5)

            # host pre-halved Wo, so mo_sb = 0.5*mo:
            # aug = h + g*mo = (h + mo_sb) + mo_sb*tanh  (plain adds/mults
            # keep GpSimd eligible -- Pool has no tensor_scalar_ptr ops)
            v_sb = csb.tile([128, HID], F32, tag="v")
            nc.gpsimd.tensor_add(h_sb[:], h_sb[:], mo_sb[:])
            nc.gpsimd.tensor_mul(v_sb[:], mo_sb[:], gb_sb[:])
            yield

            nc.vector.scalar_tensor_tensor(
                h_sb[:], h_sb[:], 0.0, v_sb[:], op0=OP.add, op1=OP.add,
                accum_out=sum_all[:, t:t + 1])
            # square's tensor output is scrap; we only keep the accumulator
            nc.scalar.activation(
                v_sb[:], h_sb[:], AF.Square, accum_out=ss_all[:, t:t + 1])

            # ---- LayerNorm finalize, per tile, VectorE only ----
            mean = stp.tile([128, 1], F32, tag="mean")
            nc.vector.tensor_scalar_mul(mean[:], sum_all[:, t:t + 1], 1.0 / HID)
            m2 = stp.tile([128, 1], F32, tag="m2")
            nc.vector.tensor_mul(m2[:], mean[:], mean[:])
            nc.vector.tensor_scalar_add(m2[:], m2[:], -LN_EPS)
            vpe = stp.tile([128, 1], F32, tag="vpe")
            nc.vector.scalar_tensor_tensor(
                vpe[:], ss_all[:, t:t + 1], 1.0 / HID, m2[:],
                op0=OP.mult, op1=OP.subtract)
            # rstd = 1/sqrt(vpe): quake init + 3 Newton iterations
            y = stp.tile([128, 1], F32, tag="y")
            yi = y[:].bitcast(I32)
            nc.vector.tensor_scalar(
                yi, vpe[:].bitcast(I32), 1, None,
                op0=OP.logical_shift_right)
            nc.vector.tensor_scalar(
                yi, yi, -RSQRT_MAGIC, -1,
                op0=OP.add, op1=OP.mult)
            yy = stp.tile([128, 1], F32, tag="yy")
            hw = stp.tile([128, 1], F32, tag="hw")
            for _ in range(3):
                nc.vector.tensor_mul(yy[:], y[:], y[:])
                nc.vector.tensor_mul(yy[:], yy[:], vpe[:])
                nc.vector.tensor_scalar(
                    hw[:], yy[:], -0.5, 1.5, op0=OP.mult, op1=OP.add)
                nc.vector.tensor_mul(y[:], y[:], hw[:])

            # yout = (aug - mean)*rstd*lng + lnb
            nc.vector.scalar_tensor_tensor(
                h_sb[:], h_sb[:], mean[:], lng_sb[:],
                op0=OP.subtract, op1=OP.mult)
            yo_sb = cstr.tile([128, HID], F32, tag="h_c")
            nc.vector.scalar_tensor_tensor(
                yo_sb[:], h_sb[:], y[:], lnb_sb[:],
                op0=OP.mult, op1=OP.add)
            nc.sync.dma_start(out_d.ap()[t], yo_sb[:])

        cgen = None
        for t in range(nt):
            cgen = c_block(t - 1) if t >= 1 else None
            b_block(t, cgen)
            if cgen is not None:
                for _ in cgen:
                    pass
        for _ in c_block(nt - 1):
            pass

    nc.compile()
    return nc


def _prep_core(hs, mk, mv, nt):
    """Host-side lossless layout prep for one core's shard."""
    hT = np.ascontiguousarray(
        hs.reshape(nt, 128, NIC, 128).transpose(0, 3, 2, 1))      # [t,p,ic,b]
    h = np.ascontiguousarray(hs.reshape(nt, 128, HID))
    mkT = np.ascontiguousarray(
        mk.reshape(nt, 128, TOPK, NIC, 128).transpose(0, 2, 4, 3, 1))
    mvT = np.ascontiguousarray(
        mv.reshape(nt, 128, TOPK, NIC, 128).transpose(0, 2, 4, 3, 1))
    return hT, h, mkT, mvT


def kernel(**inputs):
    hs = np.asarray(inputs["hidden_state"], dtype=np.float32)
    mk = np.asarray(inputs["memory_keys"], dtype=np.float32)
    mv = np.asarray(inputs["memory_values"], dtype=np.float32)

    import ml_dtypes
    bf = ml_dtypes.bfloat16
    f8 = ml_dtypes.float8_e4m3
    wq = np.ascontiguousarray(
        np.asarray(inputs["Wq"], np.float32).reshape(NIC, 128, HID).transpose(1, 0, 2)).astype(bf)
    wk = np.ascontiguousarray(
        np.asarray(inputs["Wk"], np.float32).reshape(NIC, 128, HID).transpose(1, 0, 2)).astype(f8)
    wv = np.ascontiguousarray(
        np.asarray(inputs["Wv"], np.float32).reshape(NIC, 128, HID).transpose(1, 0, 2)).astype(f8)
    # Wo pre-halved so mo_sb = 0.5*mo on chip; Wg's mo-half doubled to
    # compensate (both exact power-of-2 scalings in bf16)
    wo = np.ascontiguousarray(
        (np.asarray(inputs["Wo"], np.float32) * 0.5).reshape(NIC, 128, HID).transpose(1, 0, 2)).astype(bf)
    wg_f = np.asarray(inputs["Wg"], np.float32).copy()
    wg_f[HID:] *= 2.0
    wg = np.ascontiguousarray(
        wg_f.reshape(2 * NIC, 128, HID).transpose(1, 0, 2)).astype(bf)
    bgb = np.ascontiguousarray(
        np.broadcast_to(np.asarray(inputs["bg"], np.float32), (128, HID)))
    lng = np.ascontiguousarray(
        np.broadcast_to(np.asarray(inputs["ln_g"], np.float32).astype(bf), (128, HID)))
    lnb = np.ascontiguousarray(
        np.broadcast_to(np.asarray(inputs["ln_b"], np.float32).astype(bf), (128, HID)))
    eye = np.eye(128, dtype=np.float32).astype(bf)

    if "nc" not in _CACHE:
        _CACHE["nc"] = _build(NT)
    nc = _CACHE["nc"]

    in_maps = []
    for c in range(N_CORES):
        sl = slice(c * BC, (c + 1) * BC)
        hT, h, mkT, mvT = _prep_core(hs[sl], mk[sl], mv[sl], NT)
        in_maps.append({
            "hTb": hT.astype(bf), "h": h,
            "mkT": mkT.astype(f8), "mvT": mvT.astype(f8),
            "Wq": wq, "Wk": wk, "Wv": wv, "Wo": wo, "Wg": wg,
            "bgB": bgb, "eye": eye, "lngB": lng, "lnbB": lnb,
        })

    res = run_bass_kernel_spmd(nc, in_maps, core_ids=list(range(N_CORES)),
                               trace=TRACE)
    kernel.last_result = res
    out = np.concatenate(
        [r["out"].reshape(BC, HID) for r in res.results], axis=0)
    return out


kernel.last_result = None
